# revision 5
# baseline (speedup 1.0000x reference)
"""Trainium2 kernel for ChunkLayer boundary-token compaction.

Problem: hidden_states [B=4, L=4096, D=2048] f32, boundary_mask [B, L] bool.
Per sequence, compact rows where boundary_mask is True to the front (stable
order), truncate to K = max per-sequence count (padding past a sequence's own
count comes from the earliest dropped rows, matching the reference argsort),
and emit next_mask[b, j] = j < num_tokens[b].

Strategy: the index computation is O(B*L) bool work -> host. The memory-bound
part (gathering ~B*K rows of 8 KiB each, ~66 MiB read + 66 MiB write) runs on
8 NeuronCores: core (b, h) gathers half of sequence b's K output rows with
SWDGE dma_gather chunks (HBM -> SBUF, one descriptor per row) pipelined
against contiguous HWDGE writebacks (SBUF -> HBM).
"""

import numpy as np

import concourse.bacc as bacc
import concourse.mybir as mybir
from concourse import library_config
from concourse.bass_utils import run_bass_kernel_spmd

B, L, D = 4, 4096, 2048
N_CORES = 8
HALVES = 2  # cores per sequence
CHUNK = 512  # rows per dma_gather (hardware-validated limit is ~1024)
N_SLOTS = 3  # SBUF buffer slots for pipelining

# Stash of the most recent device-run results (exec_time_ns etc.) for test.py.
LAST_RESULT = None

_PROGRAM_CACHE = {}


def _round_up(x, m):
    return (x + m - 1) // m * m


def _build_program(num_idxs):
    """One SPMD program: gather `num_idxs` rows of D f32 from x by idx, store
    them contiguously to y. Same NEFF on all 8 cores; per-core behavior comes
    entirely from the inputs. Gather chunks rotate through N_SLOTS SBUF
    buffers so SBUF->HBM writeback overlaps the next HBM->SBUF gather."""
    assert num_idxs % 128 == 0
    icols = num_idxs // 16
    chunks = []  # (row_start, rows)
    s = 0
    while s < num_idxs:
        chunks.append((s, min(CHUNK, num_idxs - s)))
        s += CHUNK
    max_nb = CHUNK // 128

    nc = bacc.Bacc("TRN2", debug=False)
    x = nc.dram_tensor("x", [L, D], mybir.dt.float32, kind="ExternalInput")
    idx = nc.dram_tensor("idx", [128, icols], mybir.dt.int16, kind="ExternalInput")
    y = nc.dram_tensor("y", [num_idxs, D], mybir.dt.float32, kind="ExternalOutput")
    with (
        nc.Block() as block,
        nc.sbuf_tensor("dst", [128, N_SLOTS, max_nb, D], mybir.dt.float32) as dst,
        nc.sbuf_tensor("idxs_sbuf", [128, icols], mybir.dt.int16) as idxs_sbuf,
        nc.semaphore("io") as io,
        nc.semaphore("gsem") as gsem,
        nc.semaphore("ssem") as ssem,
    ):
        y_view = y.rearrange("(c p) d -> p c d", p=128)  # [128, num_idxs/128, D]

        @block.gpsimd
        def _(gpsimd):
            gpsimd.load_library(library_config.mlp)
            gpsimd.dma_start(idxs_sbuf[:], idx[:]).then_inc(io, 16)
            gpsimd.wait_ge(io, 16)
            for i, (start, rows) in enumerate(chunks):
                slot = i % N_SLOTS
                nb = rows // 128
                if i >= N_SLOTS:
                    # slot's previous contents must be stored before reuse
                    gpsimd.wait_ge(ssem, 16 * (i - N_SLOTS + 1))
                gpsimd.dma_gather(
                    dst[:, slot, :nb, :],
                    x[:],
                    idxs_sbuf[:, start // 16 : (start + rows) // 16],
                    rows,
                    rows,
                    D,
                ).then_inc(gsem, 16)

        @block.sync
        def _(sync):
            for i, (start, rows) in enumerate(chunks):
                slot = i % N_SLOTS
                nb = rows // 128
                sync.wait_ge(gsem, 16 * (i + 1))
                sync.dma_start(
                    y_view[:, start // 128 : (start + rows) // 128, :],
                    dst[:, slot, :nb, :],
                ).then_inc(ssem, 16)
            sync.wait_ge(ssem, 16 * len(chunks))

    nc.compile()
    return nc


def _wrap_idxs(idx_rows, num_idxs):
    """Pack a [num_idxs] int array into the dma_gather index layout:
    [128, num_idxs//16] int16, idx j at partition j%16 column j//16,
    replicated across the 8 GpSimd core groups of 16 partitions."""
    arr = np.asarray(idx_rows, dtype=np.int16).reshape(num_idxs // 16, 16).T
    return np.ascontiguousarray(np.tile(arr, (8, 1)))


def kernel(hidden_states, boundary_mask, mask):
    global LAST_RESULT
    hs = np.ascontiguousarray(np.asarray(hidden_states, dtype=np.float32))
    bm = np.asarray(boundary_mask).astype(bool)
    assert hs.shape == (B, L, D) and bm.shape == (B, L)

    counts = bm.sum(axis=1)
    K = int(counts.max())
    half = (K + HALVES - 1) // HALVES
    num_idxs = _round_up(max(half, 128), 128)

    # Per-sequence gather indices: kept positions first, then the earliest
    # dropped positions to fill up to K (matches the reference's stable
    # argsort of arange(L) + (~mask)*L, truncated to K).
    in_maps = []
    for b in range(B):
        kept = np.flatnonzero(bm[b])
        if kept.size < K:
            dropped = np.flatnonzero(~bm[b])[: K - kept.size]
            rows = np.concatenate([kept, dropped])
        else:
            rows = kept[:K]
        for h in range(HALVES):
            shard = rows[h * half : min((h + 1) * half, K)]
            if shard.size < num_idxs:  # pad with a benign row index
                shard = np.concatenate(
                    [shard, np.zeros(num_idxs - shard.size, dtype=shard.dtype)]
                )
            in_maps.append({"x": hs[b], "idx": _wrap_idxs(shard, num_idxs)})

    key = num_idxs
    if key not in _PROGRAM_CACHE:
        _PROGRAM_CACHE[key] = _build_program(num_idxs)
    nc = _PROGRAM_CACHE[key]

    LAST_RESULT = run_bass_kernel_spmd(nc, in_maps, core_ids=list(range(N_CORES)))
    outs = [r["y"] for r in LAST_RESULT.results]

    next_hidden_states = np.empty((B, K, D), dtype=np.float32)
    for b in range(B):
        next_hidden_states[b, :half] = outs[HALVES * b][:half]
        next_hidden_states[b, half:K] = outs[HALVES * b + 1][: K - half]
    next_mask = np.arange(K)[None, :] < counts[:, None]
    return next_hidden_states, next_mask


# revision 7
# speedup vs baseline: 1.2472x; 1.2472x over previous
"""Trainium2 kernel for ChunkLayer boundary-token compaction.

Problem: hidden_states [B=4, L=4096, D=2048] f32, boundary_mask [B, L] bool.
Per sequence, compact rows where boundary_mask is True to the front (stable
order), truncate to K = max per-sequence count (padding past a sequence's own
count comes from the earliest dropped rows, matching the reference argsort),
and emit next_mask[b, j] = j < num_tokens[b].

Strategy: the index computation is O(B*L) bool work -> host. The memory-bound
part (gathering ~B*K rows of 8 KiB each, ~66 MiB read + 66 MiB write) runs on
8 NeuronCores: core (b, h) gathers half of sequence b's K output rows with
SWDGE dma_gather chunks (HBM -> SBUF, one 8 KiB descriptor per row) pipelined
against contiguous HWDGE writebacks (SBUF -> HBM, 32 KiB descriptors into a
partition-major layout that the host un-permutes).
"""

import numpy as np

import concourse.bacc as bacc
import concourse.mybir as mybir
from concourse import library_config
from concourse.bass_utils import run_bass_kernel_spmd

B, L, D = 4, 4096, 2048
N_CORES = 8
HALVES = 2  # cores per sequence
CHUNK = 512  # max rows per dma_gather (hardware-validated limit is ~1024)
N_SLOTS = 3  # SBUF buffer slots for pipelining

# Stash of the most recent device-run results (exec_time_ns etc.) for test.py.
LAST_RESULT = None

_PROGRAM_CACHE = {}


def _round_up(x, m):
    return (x + m - 1) // m * m


def _chunks_for(num_idxs):
    """Chunk layout (row_start, rows): smallest chunk first for fast pipeline
    ramp; all sizes multiples of 128."""
    rem = num_idxs % CHUNK
    sizes = ([rem] if rem else []) + [CHUNK] * (num_idxs // CHUNK)
    out, s = [], 0
    for sz in sizes:
        out.append((s, sz))
        s += sz
    return out


def _build_program(num_idxs, valid):
    """One SPMD program: gather `valid` rows (of `num_idxs` index slots; the
    tail is -1-padded and skipped by the hardware) of D f32 from x by idx,
    store them to y in partition-major layout y[p, c, :] = row(c*128 + p).
    Same NEFF on all 8 cores; per-core behavior comes entirely from inputs."""
    assert num_idxs % 128 == 0 and 0 < valid <= num_idxs
    icols = num_idxs // 16
    chunks = _chunks_for(num_idxs)
    max_nb = max(rows for _, rows in chunks) // 128
    nbt = num_idxs // 128

    nc = bacc.Bacc("TRN2", debug=False)
    x = nc.dram_tensor("x", [L, D], mybir.dt.float32, kind="ExternalInput")
    idx = nc.dram_tensor("idx", [128, icols], mybir.dt.int16, kind="ExternalInput")
    y = nc.dram_tensor("y", [128, nbt, D], mybir.dt.float32, kind="ExternalOutput")
    # Per-chunk schedule: valid rows, full store blocks, partial partitions,
    # and cumulative gather/store counts for semaphore bookkeeping.
    infos = []
    g_cum = s_cum = 0
    for start, rows in chunks:
        v = max(0, min(valid - start, rows))
        vb, vp = v // 128, v % 128
        n_stores = (1 if vb else 0) + (1 if vp else 0)
        if v:
            g_cum += 1
            s_cum += n_stores
        infos.append((start, rows, v, vb, vp, g_cum, s_cum))
    total_stores = s_cum

    with (
        nc.Block() as block,
        nc.sbuf_tensor("dst", [128, N_SLOTS, max_nb, D], mybir.dt.float32) as dst,
        nc.sbuf_tensor("idxs_sbuf", [128, icols], mybir.dt.int16) as idxs_sbuf,
        nc.semaphore("isem") as isem,
        nc.semaphore("gsem") as gsem,
        nc.semaphore("ssem") as ssem,
    ):

        @block.gpsimd
        def _(gpsimd):
            # overlaps the idx DMA issued from sync below
            gpsimd.load_library(library_config.mlp)
            gpsimd.wait_ge(isem, 16)
            for i, (start, rows, v, vb, vp, g_cum, s_cum) in enumerate(infos):
                if v == 0:
                    continue
                slot = i % N_SLOTS
                nb = rows // 128
                if i >= N_SLOTS:
                    # slot's previous contents must be stored before reuse
                    gpsimd.wait_ge(ssem, 16 * infos[i - N_SLOTS][6])
                gpsimd.dma_gather(
                    dst[:, slot, :nb, :],
                    x[:],
                    idxs_sbuf[:, start // 16 : (start + rows) // 16],
                    rows,
                    v,
                    D,
                ).then_inc(gsem, 16)

        @block.sync
        def _(sync):
            sync.dma_start(idxs_sbuf[:], idx[:]).then_inc(isem, 16)
            for i, (start, rows, v, vb, vp, g_cum, s_cum) in enumerate(infos):
                if v == 0:
                    continue
                slot = i % N_SLOTS
                sync.wait_ge(gsem, 16 * g_cum)
                c0 = start // 128
                if vb:
                    sync.dma_start(
                        y[:, c0 : c0 + vb, :], dst[:, slot, :vb, :]
                    ).then_inc(ssem, 16)
                if vp:
                    sync.dma_start(
                        y[:vp, c0 + vb, :], dst[:vp, slot, vb, :]
                    ).then_inc(ssem, 16)
            sync.wait_ge(ssem, 16 * total_stores)

    nc.compile()
    return nc


def _wrap_idxs(idx_rows, num_idxs):
    """Pack a [num_idxs] int array into the dma_gather index layout:
    [128, num_idxs//16] int16, idx j at partition j%16 column j//16,
    replicated across the 8 GpSimd core groups of 16 partitions."""
    arr = np.asarray(idx_rows, dtype=np.int16).reshape(num_idxs // 16, 16).T
    return np.ascontiguousarray(np.tile(arr, (8, 1)))


def kernel(hidden_states, boundary_mask, mask):
    global LAST_RESULT
    hs = np.ascontiguousarray(np.asarray(hidden_states, dtype=np.float32))
    bm = np.asarray(boundary_mask).astype(bool)
    assert hs.shape == (B, L, D) and bm.shape == (B, L)

    counts = bm.sum(axis=1)
    K = int(counts.max())
    half = (K + HALVES - 1) // HALVES  # rows per core; h=1 may have one fewer
    num_idxs = _round_up(max(half, 128), 128)

    # Per-sequence gather indices: kept positions first, then the earliest
    # dropped positions to fill up to K (matches the reference's stable
    # argsort of arange(L) + (~mask)*L, truncated to K).
    in_maps = []
    for b in range(B):
        kept = np.flatnonzero(bm[b])
        if kept.size < K:
            dropped = np.flatnonzero(~bm[b])[: K - kept.size]
            rows = np.concatenate([kept, dropped])
        else:
            rows = kept[:K]
        for h in range(HALVES):
            shard = rows[h * half : min((h + 1) * half, K)].astype(np.int64)
            if shard.size < half:  # h=1 when K is odd: one extra benign row
                shard = np.concatenate(
                    [shard, np.zeros(half - shard.size, dtype=np.int64)]
                )
            pad = np.full(num_idxs - shard.size, -1, dtype=np.int64)
            in_maps.append(
                {
                    "x": hs[b],
                    "idx": _wrap_idxs(np.concatenate([shard, pad]), num_idxs),
                }
            )

    key = (num_idxs, half)
    if key not in _PROGRAM_CACHE:
        _PROGRAM_CACHE[key] = _build_program(num_idxs, half)
    nc = _PROGRAM_CACHE[key]

    LAST_RESULT = run_bass_kernel_spmd(nc, in_maps, core_ids=list(range(N_CORES)))

    next_hidden_states = np.empty((B, K, D), dtype=np.float32)
    for b in range(B):
        for h in range(HALVES):
            y_t = LAST_RESULT.results[HALVES * b + h]["y"]  # [128, nbt, D]
            rows = y_t.transpose(1, 0, 2).reshape(-1, D)
            lo = h * half
            hi = min(lo + half, K)
            next_hidden_states[b, lo:hi] = rows[: hi - lo]
    next_mask = np.arange(K)[None, :] < counts[:, None]
    return next_hidden_states, next_mask
